# revision 19
# baseline (speedup 1.0000x reference)
"""GCN layer (gnn_message_passing) Trainium2 Bass kernel.

Problem: out[b,n,:] = relu( sum_r (mean_k padded[b, idx[b,r,n,k]]) @ W_r
                            + feat[b,n] @ W_self + bias )
  B=4, N=4096, D=O=128, R=4, K=16.

Strategy: shard (batch x N-half) across 8 cores -> no collectives.
Per core (b, h):
  - DRAM table tbl[4097, 128] bf16 = [zeros; node_features[b]] (host-cast).
  - SWDGE dma_gather (transpose=False -> no xbar, safe to run queues
    concurrently) pulls neighbor rows into partitions: stream position j
    lands at [j%128, j//128, :]. Relation r's gather runs on SWDGE queue
    r; descriptor generation (the bottleneck, ~7.8 ns/idx per Q7 pair)
    runs on all four Q7 core pairs concurrently (queue q -> cores
    2q/2q+1), because the pairs run ahead through the NX instruction
    queue. Any semaphore wait on a gather parks the sequencer and kills
    that run-ahead, so chunks are small (256 nodes) and the gather pool
    is 16 tiles deep (4 full waves): tile-reuse WAR waits are already
    satisfied when dispatch reaches them.
  - Stream order per chunk: neighbor k of node (nb*128+p) at column
    nb*16+k; k-sum is a log-tree of contiguous DVE adds. The self term
    uses featT [d, n] loaded once via the HWDGE xbar transpose.
  - PE transpose (identity matmul) flips each [n,d] tile to [d,n] via
    PSUM; ACT copies back to SBUF. Then PE accumulates
    out_psum[n, o] = sum_r aggT_r.T @ (W_r/K) + selfT.T @ W_self + bias.
  - ACT applies ReLU, HWDGE stores [n, o] f32 rows.
"""

import numpy as np
import ml_dtypes

import concourse.bacc as bacc
import concourse.mybir as mybir
from concourse.tile import TileContext
from concourse.bass_utils import run_bass_kernel_spmd

B, N, D = 4, 4096, 128
R, K, O = 4, 16, 128
NCORES = 8
NH = N // 2            # nodes per core
CHUNK = 256            # nodes per steady-state chunk
# Last chunk split in two: halves the serial tail after the final
# descriptor-generation wave (transfer -> reduce -> matmul -> store).
CHUNKS = [256] * 7 + [128] * 2
NB = CHUNK // 128      # node blocks per chunk (2)
RJ = CHUNK * K         # neighbor idxs per relation-gather (4096)
NCOL = RJ // 128       # gather output columns (32)
SEG = RJ // 16         # idx cols per chunk (256)
TOTSEG = sum(c * K // 16 for c in CHUNKS)

_cache = {}


def _build():
    nc = bacc.Bacc("TRN2", num_swdge_queues=4, dynamic_dma_scratch_size=32768)
    tbl = nc.dram_tensor("tbl", [N + 1, D], mybir.dt.bfloat16, kind="ExternalInput")
    idxs = nc.dram_tensor("idxs", [128, TOTSEG], mybir.dt.int16, kind="ExternalInput")
    w = nc.dram_tensor("w", [128, R + 2, O], mybir.dt.float32, kind="ExternalInput")
    wself = nc.dram_tensor("wself", [128, O], mybir.dt.bfloat16, kind="ExternalInput")
    feat = nc.dram_tensor("feat", [NH, D], mybir.dt.bfloat16, kind="ExternalInput")
    ident = nc.dram_tensor("ident", [128, 128], mybir.dt.float32, kind="ExternalInput")
    out = nc.dram_tensor("out", [NH, O], mybir.dt.float32, kind="ExternalOutput")

    with TileContext(nc) as tc:
        with (
            tc.tile_pool(name="const", bufs=1) as cpool,
            tc.tile_pool(name="g", bufs=16) as gpool,
            tc.tile_pool(name="agg", bufs=8) as apool,
            tc.tile_pool(name="ta", bufs=2) as tapool,
            tc.tile_pool(name="tb", bufs=2) as tbpool,
            tc.tile_pool(name="tcp", bufs=2) as tcpool,
            tc.tile_pool(name="aggT", bufs=8) as atpool,
            tc.tile_pool(name="osb", bufs=3) as opool,
            tc.tile_pool(name="ps", bufs=4, space="PSUM") as pspool,
            tc.tile_pool(name="acc", bufs=2, space="PSUM") as accpool,
        ):
            # idx tile first: the first gather only waits on this load.
            idx_sb = cpool.tile([128, TOTSEG], mybir.dt.int16)
            nc.sync.dma_start(idx_sb[:], idxs[:])
            w_sb = cpool.tile([128, R + 2, O], mybir.dt.float32)
            nc.sync.dma_start(w_sb[:], w[:])
            wself_sb = cpool.tile([128, O], mybir.dt.bfloat16)
            nc.sync.dma_start(wself_sb[:], wself[:])
            id_sb = cpool.tile([128, 128], mybir.dt.float32)
            nc.sync.dma_start(id_sb[:], ident[:])
            ones = cpool.tile([1, 128], mybir.dt.float32)
            nc.vector.memset(ones[:], 1.0)
            # featT [d, n] for the self term, via the HWDGE xbar transpose
            # (safe: the non-transpose gathers never touch the xbar).
            featT = cpool.tile([128, NH], mybir.dt.bfloat16)
            nc.sync.dma_start_transpose(featT[:], feat[:])

            seg_off = 0
            node_off = 0
            for ch, csz in enumerate(CHUNKS):
                nbk = csz // 128
                rj = csz * K
                seg = rj // 16

                aggs = []
                for r in range(R):
                    g = gpool.tile([128, NCOL, D], mybir.dt.bfloat16, tag="g")
                    nc.gpsimd.dma_gather(
                        g[:, :rj // 128, :], tbl[:],
                        idx_sb[:, seg_off:seg_off + seg],
                        rj, rj, D, transpose=False, single_packet=False,
                        queue_num=r,
                    )
                    # k-sum as a log-tree of contiguous adds (full DVE rate;
                    # a strided tensor_reduce over k costs ~5x more).
                    gv = g[:, :rj // 128, :].rearrange("p (nb k) d -> p nb k d", k=K)
                    aggf = apool.tile([128, NB, D], mybir.dt.float32, tag="aggf")
                    for nb in range(nbk):
                        ta = tapool.tile([128, 8, D], mybir.dt.bfloat16, tag="ta")
                        nc.vector.tensor_add(ta[:], gv[:, nb, 0:8, :], gv[:, nb, 8:16, :])
                        tb = tbpool.tile([128, 4, D], mybir.dt.bfloat16, tag="tb")
                        nc.vector.tensor_add(tb[:], ta[:, 0:4, :], ta[:, 4:8, :])
                        tcc = tcpool.tile([128, 2, D], mybir.dt.bfloat16, tag="tc")
                        nc.vector.tensor_add(tcc[:], tb[:, 0:2, :], tb[:, 2:4, :])
                        nc.vector.tensor_add(aggf[:, nb, :], tcc[:, 0, :], tcc[:, 1, :])
                    aggs.append(aggf)

                out_sb = opool.tile([128, NB, O], mybir.dt.float32)
                for t in range(nbk):
                    # transpose [n,d] -> [d,n] through PE+PSUM, copy to SBUF
                    tts = []
                    for src in [aggs[r][:, t, :] for r in range(R)]:
                        pst = pspool.tile([128, 128], mybir.dt.float32, tag="pst")
                        nc.tensor.transpose(pst[:], src, id_sb[:])
                        tt = atpool.tile([128, 128], mybir.dt.float32, tag="tt")
                        nc.scalar.activation(
                            tt[:], pst[:], mybir.ActivationFunctionType.Copy
                        )
                        tts.append(tt)

                    ps = accpool.tile([128, O], mybir.dt.float32, tag="acc")
                    for r in range(R):
                        nc.tensor.matmul(
                            ps[:], tts[r][:], w_sb[:, r, :],
                            start=(r == 0), stop=False,
                        )
                    fsl = slice(node_off + t * 128, node_off + (t + 1) * 128)
                    nc.tensor.matmul(
                        ps[:], featT[:, fsl], wself_sb[:],
                        start=False, stop=False,
                    )
                    nc.tensor.matmul(
                        ps[:], ones[:1, :], w_sb[0:1, R + 1, :],
                        start=False, stop=True,
                    )
                    nc.scalar.activation(
                        out_sb[:, t, :], ps[:], mybir.ActivationFunctionType.Relu
                    )
                nc.sync.dma_start(
                    out[node_off:node_off + csz, :].rearrange(
                        "(t p) o -> p t o", p=128
                    ),
                    out_sb[:, :nbk, :],
                )
                seg_off += seg
                node_off += csz

    nc.compile()
    return nc


def _prep_inputs(node_features, neighbor_indices, relation_kernels, self_kernel, bias):
    """Host-side shard/layout prep. Returns per-core input maps."""
    nf = np.asarray(node_features)
    idx = np.asarray(neighbor_indices)
    in_maps = []
    tbls = []
    for b in range(B):
        t = np.zeros((N + 1, D), dtype=ml_dtypes.bfloat16)
        t[1:] = nf[b].astype(ml_dtypes.bfloat16)
        tbls.append(t)

    w = np.zeros((128, R + 2, O), dtype=np.float32)
    for r in range(R):
        w[:, r, :] = np.asarray(relation_kernels)[r] / K
    w[0, R + 1, :] = np.asarray(bias)
    wself = np.asarray(self_kernel).astype(ml_dtypes.bfloat16)
    ident = np.eye(128, dtype=np.float32)

    for c in range(NCORES):
        b, h = divmod(c, 2)
        base = h * NH
        # idx tile: partitions [32q, 32q+32) hold relation q's stream,
        # wrapped 16-wide and duplicated for both Q7 cores of pair q.
        # Stream order: neighbor k of node (nb*128+p) at position
        # (nb*16+k)*128 + p; queues 0/1 append the 128 self indices of
        # node block 0/1.
        cols = np.zeros((128, TOTSEG), dtype=np.int16)
        seg_off = 0
        lo = base
        for csz in CHUNKS:
            seg = csz * K // 16
            for r in range(R):
                blkidx = idx[b, r, lo:lo + csz, :].astype(np.int16)
                stream = blkidx.reshape(csz // 128, 128, K).transpose(0, 2, 1).reshape(-1)
                blk = stream.reshape(-1, 16).T
                cols[32 * r:32 * r + 16, seg_off:seg_off + seg] = blk
                cols[32 * r + 16:32 * r + 32, seg_off:seg_off + seg] = blk
            seg_off += seg
            lo += csz
        in_maps.append({
            "tbl": tbls[b],
            "idxs": cols,
            "w": w,
            "wself": wself,
            "feat": tbls[b][1 + base: 1 + base + NH],
            "ident": ident,
        })
    return in_maps


def _run(in_maps, **kw):
    if "nc" not in _cache:
        _cache["nc"] = _build()
    return run_bass_kernel_spmd(_cache["nc"], in_maps, core_ids=list(range(NCORES)), **kw)


def kernel(node_features, neighbor_indices, relation_kernels, self_kernel, bias):
    in_maps = _prep_inputs(node_features, neighbor_indices, relation_kernels,
                           self_kernel, bias)
    res = _run(in_maps)
    out = np.empty((B, N, O), dtype=np.float32)
    for c in range(NCORES):
        b, h = divmod(c, 2)
        out[b, h * NH:(h + 1) * NH, :] = res.results[c]["out"]
    return out
